# revision 23
# baseline (speedup 1.0000x reference)
"""Trainium2 Bass kernel for nn_DecoderLayer (dense transformer decoder layer).

Sharding: data-parallel over batch (16 batches -> 8 cores x 2 each). Each core
runs the full decoder layer on its batch slice; no collectives.

Design:
- All matmul operands are bf16 (fp32 PSUM accumulate); rel err ~7e-3 vs the
  fp32 reference. Inputs stream from DRAM in fp32 and are transposed on the
  PE (f32r, 1.5 cyc/row), with the PSUM eviction doing the bf16 downcast
  (alternating Vector/Scalar engines to balance load).
- Activations are feature-major ("xT": [feature partitions, token free]) with
  both batches merged in the free dim (T=512) so projections stream N=512.
- Cross-attention: per batch, 8 key-groups of 512 keys; K/V projection of
  group g+1 overlaps attention of group g (double-buffered k2T/vext). PV
  accumulates in PSUM within a group ([65,2,256] bank per head pair, ones
  column appended to V for the softmax denominator) and across groups into an
  f32 SBUF accumulator. Softmax: exp without max-subtraction (|s|*W^-0.5
  bounded ~2), denominators inverted via ones-broadcast matmul + wide
  reciprocal_approx_fast (no [1,N] vector ops).
- LayerNorm feature-major via ones-vector matmuls; LN3 sums folded into the
  FFN2 loop; final residual + LN3 apply in fp32. Output transposed exactly
  (fp32 PE transpose) and DMA'd out.
"""
import sys
import numpy as np

sys.path.insert(0, '/opt/trn_rl_repo')

import concourse.bass as bass  # noqa: E402
import concourse.tile as tile  # noqa: E402
from concourse import bacc, mybir  # noqa: E402
from concourse.bass_utils import run_bass_kernel_spmd  # noqa: E402
from concourse.masks import make_identity  # noqa: E402
from contextlib import ExitStack  # noqa: E402

F32 = mybir.dt.float32
F32R = mybir.dt.float32r
BF16 = mybir.dt.bfloat16
F8 = mybir.dt.float8e4
DR = mybir.MatmulPerfMode.DoubleRow
AF = mybir.ActivationFunctionType

EPS = 1e-5
N_CORES = 8


def build_decoder(nc, tc, ctx, B_loc, NQ, S, W, NH, MLP, JC=256, suffix=""):
    HD = W // NH
    assert HD == 64 and NQ % 128 == 0 and W % 512 == 0
    T = B_loc * NQ          # decoder tokens per core (both batches)
    TC = T // 128
    WC = W // 128
    MC = MLP // 128
    NQC = NQ // 128
    GK = 512                # cross-attention key-group size
    NG = S // GK
    GJS = GK // 128         # 128-key chunks per group
    SCALE = float(W) ** -0.5
    HPC = 128 // HD         # heads per feature chunk (2)

    dram = {}
    for name, shape in [
        ('query', [B_loc, NQ, W]), ('enc_mem', [B_loc, S, W]),
        ('out_pos_enc', [B_loc, NQ, W]),
        ('sa_wq', [W, W]), ('sa_wk', [W, W]), ('sa_wv', [W, W]), ('sa_wo', [W, W]),
        ('ca_wq', [W, W]), ('ca_wk', [W, W]), ('ca_wv', [W, W]), ('ca_wo', [W, W]),
        ('ffn_w1', [MLP, W]), ('ffn_b1', [MLP]), ('ffn_w2', [W, MLP]), ('ffn_b2', [W]),
        ('ln1_g', [W]), ('ln1_b', [W]), ('ln2_g', [W]), ('ln2_b', [W]),
        ('ln3_g', [W]), ('ln3_b', [W]),
    ]:
        if suffix:
            dram[name] = build_decoder._dram_cache[name]
        else:
            dram[name] = nc.dram_tensor(name, shape, F32, kind="ExternalInput")
    build_decoder._dram_cache = dict(dram)
    out_d = nc.dram_tensor("out" + suffix, [B_loc, NQ, W], F32,
                           kind="ExternalOutput")

    q_flat = dram['query'].rearrange("b n w -> (b n) w")
    pe_flat = dram['out_pos_enc'].rearrange("b n w -> (b n) w")
    m_flat = dram['enc_mem'].rearrange("b s w -> (b s) w")
    out_flat = out_d.rearrange("b n w -> (b n) w")

    # ---------------- global pools ----------------
    consts = ctx.enter_context(tc.tile_pool(name="consts", bufs=1))
    persist = ctx.enter_context(tc.tile_pool(name="persist", bufs=1))
    scratch = ctx.enter_context(tc.tile_pool(name="scratch", bufs=2))
    stat = ctx.enter_context(tc.tile_pool(name="stat", bufs=1))
    mm_ps = ctx.enter_context(tc.tile_pool(name="mm_ps", bufs=3, space="PSUM"))
    sp_ps = ctx.enter_context(tc.tile_pool(name="sp_ps", bufs=3, space="PSUM"))
    pv_ps = ctx.enter_context(tc.tile_pool(name="pv_ps", bufs=2, space="PSUM"))
    sc_ps = sp_ps
    tp_ps = sp_ps

    ident = consts.tile([128, 128], F32, tag="ident")
    make_identity(nc, ident[:])
    ident_r = consts.tile([128, 128], F32R, tag="ident_r")
    nc.vector.tensor_copy(ident_r[:], ident[:])
    ones_bf = consts.tile([128, 128], BF16, tag="ones_bf")
    nc.vector.memset(ones_bf[:], 1.0)
    ones_f = consts.tile([1, 128], F32, tag="ones_f")
    nc.vector.memset(ones_f[:], 1.0)
    eps_t = consts.tile([1, 1], F32, tag="eps")
    nc.vector.memset(eps_t[:], EPS)

    def load_col(name, n):
        """[n] f32 param vector -> [128, n/128] per-partition columns (exact)."""
        nch = n // 128
        land = scratch.tile([128, 128], F32, tag="colland", bufs=2,
                            name=name + "_land")
        nc.sync.dma_start(land[0:nch, :],
                          dram[name].rearrange("(c p) -> c p", p=128))
        pt = tp_ps.tile([128, 512], F32, tag="sp", name="pt_col")
        nc.tensor.transpose(pt[:, 0:128], land[:, 0:128], ident[:])
        t = consts.tile([128, nch], F32, tag=name, name=name + "_col")
        nc.vector.tensor_copy(t[:], pt[:, 0:nch])
        return t
    cols = {k: load_col(k, W) for k in
            ['ln1_g', 'ln1_b', 'ln2_g', 'ln2_b', 'ln3_g', 'ln3_b', 'ffn_b2']}
    b1_col = load_col('ffn_b1', MLP)

    # ---------------- helpers ----------------
    _evict_flip = [0]

    def transpose_group(dst_slice, src_slices, scale=None):
        """Transpose up to 4 [128,128] f32 blocks through one PSUM bank (f32r,
        1.5 cyc/row) and evict with a single copy that downcasts to bf16.
        Eviction alternates Vector/Scalar to balance engine load."""
        n = len(src_slices)
        pt = tp_ps.tile([128, 512], F32R, tag="sp", name="ptg")
        for i, src in enumerate(src_slices):
            nc.tensor.transpose(pt[:, i * 128:(i + 1) * 128], src,
                                ident_r[:])
        src_view = pt[:, 0:n * 128]
        if len(dst_slice.shape) == 3:
            src_view = src_view.rearrange("p (c n2) -> p c n2", n2=128)
        _evict_flip[0] ^= 1
        if scale is None:
            if _evict_flip[0]:
                nc.vector.tensor_copy(dst_slice, src_view)
            else:
                nc.scalar.activation(dst_slice, src_view, AF.Copy)
        else:
            if _evict_flip[0]:
                nc.vector.tensor_scalar_mul(dst_slice, src_view, scale)
            else:
                nc.scalar.activation(dst_slice, src_view, AF.Copy,
                                     scale=scale)

    def load_wT(pool, tag, name, w_dram, O, I, bufs=1, dtype=BF16,
                scale=None):
        """W [O, I] f32 DRAM -> W^T tile [128, I/128, O] via HWDGE fp32
        row-slab loads + PE transposes + downcast (optionally scaled)
        evictions."""
        wt = pool.tile([128, I // 128, O], dtype, tag=tag, bufs=bufs,
                       name=name)
        for co in range(O // 128):
            wr = scratch.tile([128, I], F32R, tag="wland", bufs=3,
                              name=name + "_land")
            nc.sync.dma_start(wr[:, :],
                              w_dram[co * 128:(co + 1) * 128, :].bitcast(F32R))
            for half in range(I // 512):
                transpose_group(
                    wt[:, half * 4:(half + 1) * 4, co * 128:(co + 1) * 128],
                    [wr[:, half * 512 + k * 128:half * 512 + (k + 1) * 128]
                     for k in range(4)], scale=scale)
        return wt

    def load_wT_units(pool, tag, name, w_dram, O, I, bufs=1, dtype=BF16,
                      scale=None):
        """Like load_wT but returns (wt, units): each unit emits one
        128-row slab (DMA + transposes), for interleaving with other work."""
        wt = pool.tile([128, I // 128, O], dtype, tag=tag, bufs=bufs,
                       name=name)
        units = []
        for co in range(O // 128):
            def u(co=co):
                wr = scratch.tile([128, I], F32R, tag="wland", bufs=3,
                                  name=name + "_land")
                nc.sync.dma_start(
                    wr[:, :],
                    w_dram[co * 128:(co + 1) * 128, :].bitcast(F32R))
                for half in range(I // 512):
                    transpose_group(
                        wt[:, half * 4:(half + 1) * 4,
                           co * 128:(co + 1) * 128],
                        [wr[:, half * 512 + k * 128:half * 512 + (k + 1) * 128]
                         for k in range(4)], scale=scale)
            units.append(u)
        return wt, units

    def gemm(psum, wt, oc, rhs_fn, ICn):
        for ic in range(ICn):
            nc.tensor.matmul(psum, wt[:, ic, oc * 128:(oc + 1) * 128],
                             rhs_fn(ic), start=(ic == 0), stop=(ic == ICn - 1))

    def layernorm(x_fn, n_chunks, N, g_col, b_col, out_fn,
                  pre_sums=None, f32_apply=False):
        """Feature-major LN over the partition (feature) dim, N tokens."""
        if pre_sums is None:
            ps_s = pv_ps.tile([1, N], F32, tag="pv", name="ps_s")
            ps_q = sc_ps.tile([1, N], F32, tag="sp", name="ps_q")
            for ic in range(n_chunks):
                nc.tensor.matmul(ps_s[0:1, :], ones_bf[:, 0:1], x_fn(ic),
                                 start=(ic == 0), stop=(ic == n_chunks - 1))
            for ic in range(n_chunks):
                sq = scratch.tile([128, N], BF16, tag="sq", name="sq")
                nc.vector.tensor_mul(sq[:, 0:N], x_fn(ic), x_fn(ic))
                nc.tensor.matmul(ps_q[0:1, :], ones_bf[:, 0:1], sq[:, 0:N],
                                 start=(ic == 0), stop=(ic == n_chunks - 1))
        else:
            ps_s, ps_q = pre_sums
        inv_w = 1.0 / (n_chunks * 128)
        mu = stat.tile([1, N], F32, tag="st_mu", name="mu")
        nc.scalar.activation(mu[0:1, :], ps_s[0:1, :], AF.Copy, scale=inv_w)
        ex2 = stat.tile([1, N], F32, tag="st_e", name="ex2")
        nc.scalar.activation(ex2[0:1, :], ps_q[0:1, :], AF.Copy, scale=inv_w)
        mu2 = stat.tile([1, N], F32, tag="st_x", name="mu2")
        nc.vector.tensor_mul(mu2[0:1, :], mu[0:1, :], mu[0:1, :])
        var = stat.tile([1, N], F32, tag="st_v", name="var")
        nc.vector.tensor_sub(var[0:1, :], ex2[0:1, :], mu2[0:1, :])
        sd = stat.tile([1, N], F32, tag="st_x", name="sd")
        nc.scalar.activation(sd[0:1, :], var[0:1, :], AF.Sqrt,
                             bias=eps_t[0:1, 0:1])
        ps_mu = sc_ps.tile([128, N], F32, tag="sp", name="ps_mu")
        nc.tensor.matmul(ps_mu[:, 0:N], ones_f[0:1, :], mu[0:1, :])
        ps_sd = sc_ps.tile([128, N], F32, tag="sp", name="ps_sd")
        nc.tensor.matmul(ps_sd[:, 0:N], ones_f[0:1, :], sd[0:1, :])
        rs_full = scratch.tile([128, N], F32, tag="recb", bufs=2,
                               name="rs_full")
        nc.vector.reciprocal_approx_fast(rs_full[:, 0:N], ps_sd[:, 0:N])
        for ic in range(n_chunks):
            dt = F32 if f32_apply else BF16
            xm = scratch.tile([128, N], dt, tag="xmf" if f32_apply else "sq",
                              name="xm")
            nc.vector.tensor_sub(xm[:, 0:N], x_fn(ic), ps_mu[:, 0:N])
            nc.vector.tensor_mul(xm[:, 0:N], xm[:, 0:N], rs_full[:, 0:N])
            nc.scalar.activation(out_fn(ic), xm[:, 0:N], AF.Identity,
                                 bias=b_col[:, ic:ic + 1],
                                 scale=g_col[:, ic:ic + 1])

    def normalize_batch(b, acc, oT):
        """oT[:, :, b*NQ:(b+1)*NQ] = acc PV / acc softmax sums (feature-major).

        acc: f32 [HD+1, NH, NQ]; row HD holds the softmax denominators."""
        for hp in range(NH // 2):
            fc = hp
            den2 = stat.tile([1, HPC, NQ], F32, tag="st_dn2", bufs=2,
                             name="den2")
            nc.vector.tensor_copy(den2[0:1, :, :],
                                  acc[HD:HD + 1, HPC * hp:HPC * hp + HPC, :])
            ps_db = mm_ps.tile([128, HPC, NQ], F32, tag="mm", name="ps_db")
            nc.tensor.matmul(ps_db[:, :, :], ones_f[0:1, :],
                             den2[0:1, :, :])
            rec = scratch.tile([128, HPC, NQ], F32, tag="recb", bufs=2,
                               name="rec")
            nc.vector.reciprocal_approx_fast(rec[:, :, :], ps_db[:, :, :])
            for sub in range(HPC):
                h = HPC * hp + sub
                off = sub * HD
                nc.vector.tensor_mul(
                    oT[off:off + HD, fc, b * NQ:(b + 1) * NQ],
                    acc[0:HD, h, :], rec[0:HD, sub, :])

    def attn_chains(b, qTsrc, kT, vext, a_acc, n_js, first_group, e_pool,
                    pv8=False):
        """Build attention emission chains for one key-group, batch b.

        Returns a list of (pre, post) closures per (hp, sub, jsp): `pre`
        emits the score matmuls + exp, `post` emits the PV matmul(s) (and,
        on the last chain of a head pair, the accumulator eviction).
        Interleaving independent Tensor work between pre and post hides the
        exp latency (in-order engine queues)."""
        chains = []
        state = {}
        for hp in range(NH // 2):
            for sub in range(2):
                h = 2 * hp + sub
                off = (h % HPC) * HD
                fc = h // HPC
                for jsp in range(n_js // 2):
                    def pre(hp=hp, sub=sub, jsp=jsp, h=h, off=off, fc=fc):
                        if sub == 0 and jsp == 0:
                            state[hp] = pv_ps.tile([HD + 1, 2, NQ], F32,
                                                   tag="pv", name="ps_pv")
                        ps_s = sc_ps.tile([128, 2, NQ], F32, tag="sp",
                                          name="ps_sc")
                        for j in range(2):
                            js = jsp * 2 + j
                            nc.tensor.matmul(
                                ps_s[:, j, :],
                                kT[off:off + HD, fc, js * 128:(js + 1) * 128],
                                qTsrc[off:off + HD, fc,
                                      b * NQ:(b + 1) * NQ])
                        e = e_pool.tile([128, 2, NQ], F8 if pv8 else BF16,
                                        tag="exp", bufs=2, name="e")
                        nc.scalar.activation(e[:, :, :], ps_s[:, :, :],
                                             AF.Exp, scale=SCALE)
                        state['e'] = e
                    def post(hp=hp, sub=sub, jsp=jsp, h=h):
                        ps_o = state[hp]
                        e = state['e']
                        if pv8:
                            nc.tensor.matmul(
                                ps_o[0:HD + 1, sub, :],
                                vext[:, jsp * 2:jsp * 2 + 2, h, :],
                                e[:, :, :], perf_mode=DR,
                                start=(jsp == 0),
                                stop=(jsp == n_js // 2 - 1))
                        else:
                            for j in range(2):
                                js = jsp * 2 + j
                                nc.tensor.matmul(ps_o[0:HD + 1, sub, :],
                                                 vext[:, js, h, :],
                                                 e[:, j, :],
                                                 start=(js == 0),
                                                 stop=(js == n_js - 1))
                        if sub == 1 and jsp == n_js // 2 - 1:
                            if first_group:
                                nc.vector.tensor_copy(
                                    a_acc[0:HD + 1, 2 * hp:2 * hp + 2, :],
                                    ps_o[0:HD + 1, :, :])
                            else:
                                nc.vector.tensor_add(
                                    a_acc[0:HD + 1, 2 * hp:2 * hp + 2, :],
                                    a_acc[0:HD + 1, 2 * hp:2 * hp + 2, :],
                                    ps_o[0:HD + 1, :, :])
                    chains.append((pre, post))
        return chains

    def run_interleaved(chains, fillers):
        """Emit chains, distributing filler closures between each chain's
        pre (scores+exp) and post (PV) to hide the exp latency."""
        frac_acc = 0.0
        per = len(fillers) / max(1, len(chains))
        fi = 0
        for pre, post in chains:
            pre()
            frac_acc += per
            while frac_acc >= 1.0 and fi < len(fillers):
                fillers[fi]()
                fi += 1
                frac_acc -= 1.0
            post()
        while fi < len(fillers):
            fillers[fi]()
            fi += 1

    # ================= P0 + self-attention =================
    x1T = persist.tile([128, WC, T], BF16, tag="x1T", name="x1T")
    x2T = persist.tile([128, WC, T], BF16, tag="x2T", name="x2T")
    acc = persist.tile([HD + 1, NH, NQ], F32, tag="acc", name="acc")
    acc2 = persist.tile([HD + 1, NH, NQ], F32, tag="acc2", name="acc2")

    with tc.tile_pool(name="early", bufs=1) as early, \
         tc.tile_pool(name="sa_w", bufs=2) as sa_w, \
         tc.tile_pool(name="sa", bufs=1) as sa:
        qT = early.tile([128, WC, T], BF16, tag="qT", name="qT")
        peT = early.tile([128, WC, T], BF16, tag="peT", name="peT")
        for b in range(B_loc):
            q_tm = scratch.tile([128, NQC, W], F32R, tag="tmland", bufs=2,
                                name="q_tm")
            nc.sync.dma_start(
                q_tm[:], q_flat[b * NQ:(b + 1) * NQ, :].rearrange(
                    "(c p) w -> p c w", p=128).bitcast(F32R))
            p_tm = scratch.tile([128, NQC, W], F32R, tag="tmland", bufs=2,
                                name="p_tm")
            nc.scalar.dma_start(
                p_tm[:], pe_flat[b * NQ:(b + 1) * NQ, :].rearrange(
                    "(c p) w -> p c w", p=128).bitcast(F32R))
            for fc in range(WC):
                t0 = b * NQ
                transpose_group(
                    qT[:, fc, t0:t0 + NQ],
                    [q_tm[:, tcx, fc * 128:(fc + 1) * 128]
                     for tcx in range(NQC)])
                transpose_group(
                    peT[:, fc, t0:t0 + NQ],
                    [p_tm[:, tcx, fc * 128:(fc + 1) * 128]
                     for tcx in range(NQC)])
        qkT = sa.tile([128, WC, T], BF16, tag="bigA", name="qkT")
        for fc in range(WC):
            nc.vector.tensor_add(qkT[:, fc, :], qT[:, fc, :], peT[:, fc, :])

        # -------- SA projections (batch-merged, N=T) --------
        wqt = load_wT(sa_w, "wt", "sa_wq_t", dram['sa_wq'], W, W, bufs=2)
        qsaT = sa.tile([128, WC, T], BF16, tag="qsaT", name="qsaT")
        for oc in range(WC):
            ps = mm_ps.tile([128, T], F32, tag="mm", name="ps_q")
            gemm(ps[:, 0:T], wqt, oc, lambda ic: qkT[:, ic, :], WC)
            nc.vector.tensor_copy(qsaT[:, oc, :], ps[:, 0:T])
        wkt = load_wT(sa_w, "wt", "sa_wk_t", dram['sa_wk'], W, W, bufs=2)
        ksaT = sa.tile([128, WC, T], BF16, tag="ksaT", name="ksaT")
        for oc in range(WC):
            ps = mm_ps.tile([128, T], F32, tag="mm", name="ps_k")
            gemm(ps[:, 0:T], wkt, oc, lambda ic: qkT[:, ic, :], WC)
            nc.vector.tensor_copy(ksaT[:, oc, :], ps[:, 0:T])
        wvt = load_wT(sa_w, "wt", "sa_wv_t", dram['sa_wv'], W, W, bufs=2)
        vext_sa = sa.tile([128, TC, NH, HD + 1], BF16, tag="vext",
                          name="vext_sa")
        nc.vector.memset(vext_sa[:, :, :, HD], 1.0)
        for tcx in range(TC):
            for oh in range(W // 512):
                ps = mm_ps.tile([128, 512], F32, tag="mm", name="ps_v")
                for ic in range(WC):
                    nc.tensor.matmul(
                        ps[:, 0:512],
                        qT[:, ic, tcx * 128:(tcx + 1) * 128],
                        wvt[:, ic, oh * 512:(oh + 1) * 512],
                        start=(ic == 0), stop=(ic == WC - 1))
                nh0 = oh * (512 // HD)
                nc.scalar.activation(
                    vext_sa[:, tcx, nh0:nh0 + 512 // HD, 0:HD],
                    ps[:, 0:512].rearrange("p (h d) -> p h d", d=HD),
                    AF.Copy)
        wot, wot_units = load_wT_units(sa_w, "wt", "sa_wo_t", dram['sa_wo'],
                                       W, W, bufs=2)
        wqt2, wq2_units = load_wT_units(sa_w, "wt", "ca_wq_t",
                                        dram['ca_wq'], W, W, bufs=2)

        # -------- SA attention (batches interleaved) + O proj + LN1 --------
        osaT = sa.tile([128, WC, T], BF16, tag="bigA", name="osaT")
        accs = [acc, acc2]
        blists = [attn_chains(b, qsaT, ksaT[:, :, b * NQ:(b + 1) * NQ],
                              vext_sa[:, b * NQC:(b + 1) * NQC, :, :],
                              accs[b], NQC, True, sa)
                  for b in range(B_loc)]
        chains = [c for pair in zip(*blists) for c in pair]
        run_interleaved(chains, wot_units + wq2_units)
        for b in range(B_loc):
            normalize_batch(b, accs[b], osaT)
        x1pre = sa.tile([128, WC, T], BF16, tag="bigB", name="x1pre")
        for oc in range(WC):
            ps = mm_ps.tile([128, T], F32, tag="mm", name="ps_o")
            gemm(ps[:, 0:T], wot, oc, lambda ic: osaT[:, ic, :], WC)
            nc.vector.tensor_add(x1pre[:, oc, :], ps[:, 0:T], qT[:, oc, :])
        layernorm(lambda ic: x1pre[:, ic, :], WC, T,
                  cols['ln1_g'], cols['ln1_b'],
                  lambda ic: x1T[:, ic, :])

        # -------- CA Q projection (needs peT before it dies) --------
        x1pT = sa.tile([128, WC, T], BF16, tag="bigB", name="x1pT")
        for fc in range(WC):
            nc.vector.tensor_add(x1pT[:, fc, :], x1T[:, fc, :],
                                 peT[:, fc, :])
        q2T = persist.tile([128, WC, T], BF16, tag="q2T", name="q2T")
        for oc in range(WC):
            ps = mm_ps.tile([128, T], F32, tag="mm", name="ps_q2")
            gemm(ps[:, 0:T], wqt2, oc, lambda ic: x1pT[:, ic, :], WC)
            nc.vector.tensor_copy(q2T[:, oc, :], ps[:, 0:T])

    # ================= cross-attention =================
    with tc.tile_pool(name="ca_w", bufs=1) as ca_w, \
         tc.tile_pool(name="ca", bufs=1) as ca, \
         tc.tile_pool(name="ca_g", bufs=2) as ca_g:
        wkt2 = load_wT(ca_w, "wtk2", "ca_wk_t", dram['ca_wk'], W, W,
                       dtype=F8, scale=16.0)
        wvt2 = load_wT(ca_w, "wtv2", "ca_wv_t", dram['ca_wv'], W, W,
                       dtype=F8, scale=16.0)

        ocaT = ca.tile([128, WC, T], BF16, tag="ocaT", name="ocaT")

        def build_proj(b, g):
            """Allocate group tiles and return (k2T, vext, units) where each
            unit emits one slice of the K/V projection pipeline."""
            k2T = ca_g.tile([128, WC, GK], BF16, tag="k2T", name="k2T")
            vext = ca_g.tile([128, GJS, NH, HD + 1], F8, tag="vext",
                             name="vext_ca")
            mT = ca.tile([128, WC, GK], F8, tag="mT8", bufs=2, name="mT")
            pst = {}
            units = [lambda: nc.vector.memset(vext[:, :, :, HD], 4.0)]
            for half in range(GJS // 2):
                def u_dma(half=half):
                    m_tm = scratch.tile([128, 2, W], F32R, tag="tmland",
                                        bufs=2, name="m_tm")
                    tok0 = b * S + g * GK + half * 256
                    nc.sync.dma_start(
                        m_tm[:], m_flat[tok0:tok0 + 256, :].rearrange(
                            "(c p) w -> p c w", p=128).bitcast(F32R))
                    pst[half] = m_tm
                units.append(u_dma)
                for fc in range(WC):
                    def u_tp(half=half, fc=fc):
                        transpose_group(
                            mT[:, fc, half * 256:(half + 1) * 256],
                            [pst[half][:, tcx, fc * 128:(fc + 1) * 128]
                             for tcx in range(2)])
                    units.append(u_tp)
            for oc in range(WC):
                def u_k(oc=oc):
                    ps = mm_ps.tile([128, GK], F32, tag="mm", name="ps_k2")
                    for icp in range(WC // 2):
                        nc.tensor.matmul(
                            ps[:, 0:GK],
                            wkt2[:, 2 * icp:2 * icp + 2,
                                 oc * 128:(oc + 1) * 128],
                            mT[:, 2 * icp:2 * icp + 2, :], perf_mode=DR,
                            start=(icp == 0), stop=(icp == WC // 2 - 1))
                    nc.vector.tensor_scalar_mul(k2T[:, oc, :], ps[:, 0:GK],
                                                1.0 / 16.0)
                units.append(u_k)
            for tch in range(GJS):
                for oh in range(W // 512):
                    def u_v(tch=tch, oh=oh):
                        ps = mm_ps.tile([128, 512], F32, tag="mm",
                                        name="ps_v2")
                        for icp in range(WC // 2):
                            nc.tensor.matmul(
                                ps[:, 0:512],
                                mT[:, 2 * icp:2 * icp + 2,
                                   tch * 128:(tch + 1) * 128],
                                wvt2[:, 2 * icp:2 * icp + 2,
                                     oh * 512:(oh + 1) * 512], perf_mode=DR,
                                start=(icp == 0), stop=(icp == WC // 2 - 1))
                        nh0 = oh * (512 // HD)
                        nc.vector.tensor_scalar_mul(
                            vext[:, tch, nh0:nh0 + 512 // HD, 0:HD],
                            ps[:, 0:512].rearrange("p (h d) -> p h d",
                                                   d=HD), 0.25)
                    units.append(u_v)
            return k2T, vext, units

        pending = None
        for b in range(B_loc):
            for g in range(NG):
                k2T, vext, units = build_proj(b, g)
                if pending is None:
                    for u in units:
                        u()
                else:
                    pb, pg, pk2T, pvext = pending
                    ch = attn_chains(pb, q2T, pk2T, pvext, acc, GJS,
                                     pg == 0, ca, pv8=True)
                    run_interleaved(ch, units)
                    if pg == NG - 1:
                        normalize_batch(pb, acc, ocaT)
                pending = (b, g, k2T, vext)
        pb, pg, pk2T, pvext = pending
        ch = attn_chains(pb, q2T, pk2T, pvext, acc, GJS, pg == 0, ca,
                         pv8=True)
        run_interleaved(ch, [])
        normalize_batch(pb, acc, ocaT)

        # -------- CA O proj + LN2 --------
        wot2 = load_wT(ca_w, "wtk2", "ca_wo_t", dram['ca_wo'], W, W)
        x2pre = ca.tile([128, WC, 512], BF16, tag="mT", bufs=2, name="x2pre")
        for oc in range(WC):
            ps = mm_ps.tile([128, T], F32, tag="mm", name="ps_o2")
            gemm(ps[:, 0:T], wot2, oc, lambda ic: ocaT[:, ic, :], WC)
            nc.vector.tensor_add(x2pre[:, oc, :], ps[:, 0:T], x1T[:, oc, :])
        layernorm(lambda ic: x2pre[:, ic, :], WC, T,
                  cols['ln2_g'], cols['ln2_b'],
                  lambda ic: x2T[:, ic, :])

    # ================= FFN =================
    with tc.tile_pool(name="ffn", bufs=1) as ffn:
        hT = ffn.tile([128, MC, T], BF16, tag="hT", name="hT")
        for oc in range(MC):
            w1t = ffn.tile([128, WC, 128], BF16, tag="w1t", bufs=3,
                           name="w1t")
            wr = scratch.tile([128, W], F32R, tag="wland", bufs=3, name="wr1")
            nc.sync.dma_start(
                wr[:], dram['ffn_w1'][oc * 128:(oc + 1) * 128, :]
                .bitcast(F32R))
            for half in range(W // 512):
                transpose_group(
                    w1t[:, half * 4:(half + 1) * 4, :],
                    [wr[:, half * 512 + k * 128:half * 512 + (k + 1) * 128]
                     for k in range(4)])
            ps = mm_ps.tile([128, T], F32, tag="mm", name="ps_h")
            for ic in range(WC):
                nc.tensor.matmul(ps[:, 0:T], w1t[:, ic, :], x2T[:, ic, :],
                                 start=(ic == 0), stop=(ic == WC - 1))
            nc.scalar.activation(hT[:, oc, :], ps[:, 0:T], AF.Relu,
                                 bias=b1_col[:, oc:oc + 1])
        x3pre = ffn.tile([128, WC, T], F32, tag="x3pre", name="x3pre")
        ps_s3 = pv_ps.tile([1, T], F32, tag="pv", name="ps_s3")
        ps_q3 = pv_ps.tile([1, T], F32, tag="pv", name="ps_q3")
        for oc in range(WC):
            w2t = ffn.tile([128, MC, 128], BF16, tag="w2t", bufs=2,
                           name="w2t")
            for piece in range(MLP // 1024):
                wr = scratch.tile([128, 1024], F32R, tag="wland", bufs=3,
                                  name="wr2")
                nc.sync.dma_start(
                    wr[:], dram['ffn_w2'][oc * 128:(oc + 1) * 128,
                                          piece * 1024:(piece + 1) * 1024]
                    .bitcast(F32R))
                for hh in range(2):
                    half = piece * 2 + hh
                    transpose_group(
                        w2t[:, half * 4:(half + 1) * 4, :],
                        [wr[:, hh * 512 + k * 128:hh * 512 + (k + 1) * 128]
                         for k in range(4)])
            ps = mm_ps.tile([128, T], F32, tag="mm", name="ps_f")
            for ic in range(MC):
                nc.tensor.matmul(ps[:, 0:T], w2t[:, ic, :], hT[:, ic, :],
                                 start=(ic == 0), stop=(ic == MC - 1))
            tmp = scratch.tile([128, T], F32, tag="ftmp", name="f_tmp")
            nc.scalar.activation(tmp[:, 0:T], ps[:, 0:T], AF.Identity,
                                 bias=cols['ffn_b2'][:, oc:oc + 1])
            nc.vector.tensor_add(x3pre[:, oc, :], tmp[:, 0:T], x2T[:, oc, :])
            # fold LN3 partition sums into this loop (bf16 shadow for matmul)
            x3b = scratch.tile([128, T], BF16, tag="sq3", name="x3b")
            nc.vector.tensor_copy(x3b[:, 0:T], x3pre[:, oc, :])
            nc.tensor.matmul(ps_s3[0:1, :], ones_bf[:, 0:1], x3b[:, 0:T],
                             start=(oc == 0), stop=(oc == WC - 1))
            sq = scratch.tile([128, T], BF16, tag="sq3", name="sq3")
            nc.vector.tensor_mul(sq[:, 0:T], x3b[:, 0:T], x3b[:, 0:T])
            nc.tensor.matmul(ps_q3[0:1, :], ones_bf[:, 0:1], sq[:, 0:T],
                             start=(oc == 0), stop=(oc == WC - 1))
        x3T = ffn.tile([128, WC, T], F32, tag="x3T", name="x3T")
        layernorm(lambda ic: x3pre[:, ic, :], WC, T,
                  cols['ln3_g'], cols['ln3_b'],
                  lambda ic: x3T[:, ic, :], pre_sums=(ps_s3, ps_q3),
                  f32_apply=True)
        for tcx in range(TC):
            o_tm = ffn.tile([128, W], F32, tag="o_tm", bufs=1, name="o_tm")
            for g in range(WC // 4):
                pt = tp_ps.tile([128, 512], F32, tag="sp", name="pt_out")
                for k in range(4):
                    nc.tensor.transpose(
                        pt[:, k * 128:(k + 1) * 128],
                        x3T[:, g * 4 + k, tcx * 128:(tcx + 1) * 128],
                        ident[:])
                nc.vector.tensor_copy(o_tm[:, g * 512:(g + 1) * 512],
                                      pt[:, 0:512])
            nc.sync.dma_start(out_flat[tcx * 128:(tcx + 1) * 128, :], o_tm[:])

    return out_d


_PROGRAM_CACHE = {}


def _get_program(B_loc, NQ, S, W, NH, MLP, JC=256, repeat=1):
    key = (B_loc, NQ, S, W, NH, MLP, JC, repeat)
    if key not in _PROGRAM_CACHE:
        nc = bacc.Bacc("TRN2", target_bir_lowering=False, debug=False)
        with tile.TileContext(nc) as tc, \
             nc.allow_low_precision(reason="bf16 matmul pipeline"):
            for r in range(repeat):
                with ExitStack() as ctx:
                    build_decoder(nc, tc, ctx, B_loc, NQ, S, W, NH, MLP, JC,
                                  suffix=("" if r == 0 else f"_r{r}"))
        nc.compile()
        _PROGRAM_CACHE[key] = nc
    return _PROGRAM_CACHE[key]


def kernel(**inputs):
    B, NQ, W = inputs['query'].shape
    S = inputs['enc_mem'].shape[1]
    MLP = inputs['ffn_w1'].shape[0]
    NH = 16
    assert B % N_CORES == 0
    B_loc = B // N_CORES

    nc = _get_program(B_loc, NQ, S, W, NH, MLP)

    shard_names = {'query', 'enc_mem', 'out_pos_enc'}
    in_maps = []
    for c in range(N_CORES):
        m = {}
        for k, v in inputs.items():
            v = np.ascontiguousarray(np.asarray(v, dtype=np.float32))
            if k in shard_names:
                m[k] = np.ascontiguousarray(v[c * B_loc:(c + 1) * B_loc])
            else:
                m[k] = v
        in_maps.append(m)

    res = run_bass_kernel_spmd(nc, in_maps, list(range(N_CORES)))
    return np.concatenate([res.results[c]["out"] for c in range(N_CORES)],
                          axis=0)


# revision 24
# speedup vs baseline: 1.0112x; 1.0112x over previous
"""Trainium2 Bass kernel for nn_DecoderLayer (dense transformer decoder layer).

Sharding: data-parallel over batch (16 batches -> 8 cores x 2 each). Each core
runs the full decoder layer on its batch slice; no collectives.

Design:
- All matmul operands are bf16 (fp32 PSUM accumulate); rel err ~7e-3 vs the
  fp32 reference. Inputs stream from DRAM in fp32 and are transposed on the
  PE (f32r, 1.5 cyc/row), with the PSUM eviction doing the bf16 downcast
  (alternating Vector/Scalar engines to balance load).
- Activations are feature-major ("xT": [feature partitions, token free]) with
  both batches merged in the free dim (T=512) so projections stream N=512.
- Cross-attention: per batch, 8 key-groups of 512 keys; K/V projection of
  group g+1 overlaps attention of group g (double-buffered k2T/vext). PV
  accumulates in PSUM within a group ([65,2,256] bank per head pair, ones
  column appended to V for the softmax denominator) and across groups into an
  f32 SBUF accumulator. Softmax: exp without max-subtraction (|s|*W^-0.5
  bounded ~2), denominators inverted via ones-broadcast matmul + wide
  reciprocal_approx_fast (no [1,N] vector ops).
- LayerNorm feature-major via ones-vector matmuls; LN3 sums folded into the
  FFN2 loop; final residual + LN3 apply in fp32. Output transposed exactly
  (fp32 PE transpose) and DMA'd out.
"""
import sys
import numpy as np

sys.path.insert(0, '/opt/trn_rl_repo')

import concourse.bass as bass  # noqa: E402
import concourse.tile as tile  # noqa: E402
from concourse import bacc, mybir  # noqa: E402
from concourse.bass_utils import run_bass_kernel_spmd  # noqa: E402
from concourse.masks import make_identity  # noqa: E402
from contextlib import ExitStack  # noqa: E402

F32 = mybir.dt.float32
F32R = mybir.dt.float32r
BF16 = mybir.dt.bfloat16
F8 = mybir.dt.float8e4
DR = mybir.MatmulPerfMode.DoubleRow
AF = mybir.ActivationFunctionType

EPS = 1e-5
N_CORES = 8


def build_decoder(nc, tc, ctx, B_loc, NQ, S, W, NH, MLP, JC=256, suffix=""):
    HD = W // NH
    assert HD == 64 and NQ % 128 == 0 and W % 512 == 0
    T = B_loc * NQ          # decoder tokens per core (both batches)
    TC = T // 128
    WC = W // 128
    MC = MLP // 128
    NQC = NQ // 128
    GK = 512                # cross-attention key-group size
    NG = S // GK
    GJS = GK // 128         # 128-key chunks per group
    SCALE = float(W) ** -0.5
    HPC = 128 // HD         # heads per feature chunk (2)

    dram = {}
    for name, shape in [
        ('query', [B_loc, NQ, W]), ('enc_mem', [B_loc, S, W]),
        ('out_pos_enc', [B_loc, NQ, W]),
        ('sa_wq', [W, W]), ('sa_wk', [W, W]), ('sa_wv', [W, W]), ('sa_wo', [W, W]),
        ('ca_wq', [W, W]), ('ca_wk', [W, W]), ('ca_wv', [W, W]), ('ca_wo', [W, W]),
        ('ffn_w1', [MLP, W]), ('ffn_b1', [MLP]), ('ffn_w2', [W, MLP]), ('ffn_b2', [W]),
        ('ln1_g', [W]), ('ln1_b', [W]), ('ln2_g', [W]), ('ln2_b', [W]),
        ('ln3_g', [W]), ('ln3_b', [W]),
    ]:
        if suffix:
            dram[name] = build_decoder._dram_cache[name]
        else:
            dram[name] = nc.dram_tensor(name, shape, F32, kind="ExternalInput")
    build_decoder._dram_cache = dict(dram)
    out_d = nc.dram_tensor("out" + suffix, [B_loc, NQ, W], F32,
                           kind="ExternalOutput")

    q_flat = dram['query'].rearrange("b n w -> (b n) w")
    pe_flat = dram['out_pos_enc'].rearrange("b n w -> (b n) w")
    m_flat = dram['enc_mem'].rearrange("b s w -> (b s) w")
    out_flat = out_d.rearrange("b n w -> (b n) w")

    # ---------------- global pools ----------------
    consts = ctx.enter_context(tc.tile_pool(name="consts", bufs=1))
    persist = ctx.enter_context(tc.tile_pool(name="persist", bufs=1))
    scratch = ctx.enter_context(tc.tile_pool(name="scratch", bufs=2))
    stat = ctx.enter_context(tc.tile_pool(name="stat", bufs=1))
    mm_ps = ctx.enter_context(tc.tile_pool(name="mm_ps", bufs=2, space="PSUM"))
    sc_ps = ctx.enter_context(tc.tile_pool(name="sc_ps", bufs=2, space="PSUM"))
    pv_ps = ctx.enter_context(tc.tile_pool(name="pv_ps", bufs=2, space="PSUM"))
    tp_ps = ctx.enter_context(tc.tile_pool(name="tp_ps", bufs=2, space="PSUM"))

    ident = consts.tile([128, 128], F32, tag="ident")
    make_identity(nc, ident[:])
    ident_r = consts.tile([128, 128], F32R, tag="ident_r")
    nc.vector.tensor_copy(ident_r[:], ident[:])
    ones_bf = consts.tile([128, 128], BF16, tag="ones_bf")
    nc.vector.memset(ones_bf[:], 1.0)
    ones_f = consts.tile([1, 128], F32, tag="ones_f")
    nc.vector.memset(ones_f[:], 1.0)
    eps_t = consts.tile([1, 1], F32, tag="eps")
    nc.vector.memset(eps_t[:], EPS)

    def load_col(name, n):
        """[n] f32 param vector -> [128, n/128] per-partition columns (exact)."""
        nch = n // 128
        land = scratch.tile([128, 128], F32, tag="colland", bufs=2,
                            name=name + "_land")
        nc.sync.dma_start(land[0:nch, :],
                          dram[name].rearrange("(c p) -> c p", p=128))
        pt = tp_ps.tile([128, 512], F32, tag="tp", name="pt_col")
        nc.tensor.transpose(pt[:, 0:128], land[:, 0:128], ident[:])
        t = consts.tile([128, nch], F32, tag=name, name=name + "_col")
        nc.vector.tensor_copy(t[:], pt[:, 0:nch])
        return t
    cols = {k: load_col(k, W) for k in
            ['ln1_g', 'ln1_b', 'ln2_g', 'ln2_b', 'ln3_g', 'ln3_b', 'ffn_b2']}
    b1_col = load_col('ffn_b1', MLP)

    # ---------------- helpers ----------------
    _evict_flip = [0]

    def transpose_group(dst_slice, src_slices, scale=None):
        """Transpose up to 4 [128,128] f32 blocks through one PSUM bank (f32r,
        1.5 cyc/row) and evict with a single copy that downcasts to bf16.
        Eviction alternates Vector/Scalar to balance engine load."""
        n = len(src_slices)
        pt = tp_ps.tile([128, 512], F32R, tag="tp", name="ptg")
        for i, src in enumerate(src_slices):
            nc.tensor.transpose(pt[:, i * 128:(i + 1) * 128], src,
                                ident_r[:])
        src_view = pt[:, 0:n * 128]
        if len(dst_slice.shape) == 3:
            src_view = src_view.rearrange("p (c n2) -> p c n2", n2=128)
        _evict_flip[0] ^= 1
        if scale is None:
            if _evict_flip[0]:
                nc.vector.tensor_copy(dst_slice, src_view)
            else:
                nc.scalar.activation(dst_slice, src_view, AF.Copy)
        else:
            if _evict_flip[0]:
                nc.vector.tensor_scalar_mul(dst_slice, src_view, scale)
            else:
                nc.scalar.activation(dst_slice, src_view, AF.Copy,
                                     scale=scale)

    def load_wT(pool, tag, name, w_dram, O, I, bufs=1, dtype=BF16,
                scale=None):
        """W [O, I] f32 DRAM -> W^T tile [128, I/128, O] via HWDGE fp32
        row-slab loads + PE transposes + downcast (optionally scaled)
        evictions."""
        wt = pool.tile([128, I // 128, O], dtype, tag=tag, bufs=bufs,
                       name=name)
        for co in range(O // 128):
            wr = scratch.tile([128, I], F32R, tag="wland", bufs=3,
                              name=name + "_land")
            nc.sync.dma_start(wr[:, :],
                              w_dram[co * 128:(co + 1) * 128, :].bitcast(F32R))
            for half in range(I // 512):
                transpose_group(
                    wt[:, half * 4:(half + 1) * 4, co * 128:(co + 1) * 128],
                    [wr[:, half * 512 + k * 128:half * 512 + (k + 1) * 128]
                     for k in range(4)], scale=scale)
        return wt

    def load_wT_units(pool, tag, name, w_dram, O, I, bufs=1, dtype=BF16,
                      scale=None):
        """Like load_wT but returns (wt, units): each unit emits one
        128-row slab (DMA + transposes), for interleaving with other work."""
        wt = pool.tile([128, I // 128, O], dtype, tag=tag, bufs=bufs,
                       name=name)
        units = []
        for co in range(O // 128):
            def u(co=co):
                wr = scratch.tile([128, I], F32R, tag="wland", bufs=3,
                                  name=name + "_land")
                nc.sync.dma_start(
                    wr[:, :],
                    w_dram[co * 128:(co + 1) * 128, :].bitcast(F32R))
                for half in range(I // 512):
                    transpose_group(
                        wt[:, half * 4:(half + 1) * 4,
                           co * 128:(co + 1) * 128],
                        [wr[:, half * 512 + k * 128:half * 512 + (k + 1) * 128]
                         for k in range(4)], scale=scale)
            units.append(u)
        return wt, units

    def gemm(psum, wt, oc, rhs_fn, ICn):
        for ic in range(ICn):
            nc.tensor.matmul(psum, wt[:, ic, oc * 128:(oc + 1) * 128],
                             rhs_fn(ic), start=(ic == 0), stop=(ic == ICn - 1))

    def layernorm(x_fn, n_chunks, N, g_col, b_col, out_fn,
                  pre_sums=None, f32_apply=False):
        """Feature-major LN over the partition (feature) dim, N tokens."""
        if pre_sums is None:
            ps_s = pv_ps.tile([1, N], F32, tag="pv", name="ps_s")
            ps_q = sc_ps.tile([1, N], F32, tag="sc", name="ps_q")
            for ic in range(n_chunks):
                nc.tensor.matmul(ps_s[0:1, :], ones_bf[:, 0:1], x_fn(ic),
                                 start=(ic == 0), stop=(ic == n_chunks - 1))
            for ic in range(n_chunks):
                sq = scratch.tile([128, N], BF16, tag="sq", name="sq")
                nc.vector.tensor_mul(sq[:, 0:N], x_fn(ic), x_fn(ic))
                nc.tensor.matmul(ps_q[0:1, :], ones_bf[:, 0:1], sq[:, 0:N],
                                 start=(ic == 0), stop=(ic == n_chunks - 1))
        else:
            ps_s, ps_q = pre_sums
        inv_w = 1.0 / (n_chunks * 128)
        mu = stat.tile([1, N], F32, tag="st_mu", name="mu")
        nc.scalar.activation(mu[0:1, :], ps_s[0:1, :], AF.Copy, scale=inv_w)
        ex2 = stat.tile([1, N], F32, tag="st_e", name="ex2")
        nc.scalar.activation(ex2[0:1, :], ps_q[0:1, :], AF.Copy, scale=inv_w)
        mu2 = stat.tile([1, N], F32, tag="st_x", name="mu2")
        nc.vector.tensor_mul(mu2[0:1, :], mu[0:1, :], mu[0:1, :])
        var = stat.tile([1, N], F32, tag="st_v", name="var")
        nc.vector.tensor_sub(var[0:1, :], ex2[0:1, :], mu2[0:1, :])
        sd = stat.tile([1, N], F32, tag="st_x", name="sd")
        nc.scalar.activation(sd[0:1, :], var[0:1, :], AF.Sqrt,
                             bias=eps_t[0:1, 0:1])
        ps_mu = sc_ps.tile([128, N], F32, tag="sc", name="ps_mu")
        nc.tensor.matmul(ps_mu[:, 0:N], ones_f[0:1, :], mu[0:1, :])
        ps_sd = sc_ps.tile([128, N], F32, tag="sc", name="ps_sd")
        nc.tensor.matmul(ps_sd[:, 0:N], ones_f[0:1, :], sd[0:1, :])
        rs_full = scratch.tile([128, N], F32, tag="recb", bufs=2,
                               name="rs_full")
        nc.vector.reciprocal_approx_fast(rs_full[:, 0:N], ps_sd[:, 0:N])
        for ic in range(n_chunks):
            dt = F32 if f32_apply else BF16
            xm = scratch.tile([128, N], dt, tag="xmf" if f32_apply else "sq",
                              name="xm")
            nc.vector.tensor_sub(xm[:, 0:N], x_fn(ic), ps_mu[:, 0:N])
            nc.vector.tensor_mul(xm[:, 0:N], xm[:, 0:N], rs_full[:, 0:N])
            nc.scalar.activation(out_fn(ic), xm[:, 0:N], AF.Identity,
                                 bias=b_col[:, ic:ic + 1],
                                 scale=g_col[:, ic:ic + 1])

    def normalize_batch(b, acc, oT):
        """oT[:, :, b*NQ:(b+1)*NQ] = acc PV / acc softmax sums (feature-major).

        acc: f32 [HD+1, NH, NQ]; row HD holds the softmax denominators."""
        for hp in range(NH // 2):
            fc = hp
            den2 = stat.tile([1, HPC, NQ], F32, tag="st_dn2", bufs=2,
                             name="den2")
            nc.vector.tensor_copy(den2[0:1, :, :],
                                  acc[HD:HD + 1, HPC * hp:HPC * hp + HPC, :])
            ps_db = mm_ps.tile([128, HPC, NQ], F32, tag="mm", name="ps_db")
            nc.tensor.matmul(ps_db[:, :, :], ones_f[0:1, :],
                             den2[0:1, :, :])
            rec = scratch.tile([128, HPC, NQ], F32, tag="recb", bufs=2,
                               name="rec")
            nc.vector.reciprocal_approx_fast(rec[:, :, :], ps_db[:, :, :])
            for sub in range(HPC):
                h = HPC * hp + sub
                off = sub * HD
                nc.vector.tensor_mul(
                    oT[off:off + HD, fc, b * NQ:(b + 1) * NQ],
                    acc[0:HD, h, :], rec[0:HD, sub, :])

    def attn_chains(b, qTsrc, kT, vext, a_acc, n_js, first_group, e_pool,
                    pv8=False):
        """Build attention emission chains for one key-group, batch b.

        Returns a list of (pre, post) closures per (hp, sub, jsp): `pre`
        emits the score matmuls + exp, `post` emits the PV matmul(s) (and,
        on the last chain of a head pair, the accumulator eviction).
        Interleaving independent Tensor work between pre and post hides the
        exp latency (in-order engine queues)."""
        chains = []
        state = {}
        for hp in range(NH // 2):
            for sub in range(2):
                h = 2 * hp + sub
                off = (h % HPC) * HD
                fc = h // HPC
                for jsp in range(n_js // 2):
                    def pre(hp=hp, sub=sub, jsp=jsp, h=h, off=off, fc=fc):
                        if sub == 0 and jsp == 0:
                            state[hp] = pv_ps.tile([HD + 1, 2, NQ], F32,
                                                   tag="pv", name="ps_pv")
                        ps_s = sc_ps.tile([128, 2, NQ], F32, tag="sc",
                                          name="ps_sc")
                        for j in range(2):
                            js = jsp * 2 + j
                            nc.tensor.matmul(
                                ps_s[:, j, :],
                                kT[off:off + HD, fc, js * 128:(js + 1) * 128],
                                qTsrc[off:off + HD, fc,
                                      b * NQ:(b + 1) * NQ])
                        e = e_pool.tile([128, 2, NQ], F8 if pv8 else BF16,
                                        tag="exp", bufs=2, name="e")
                        nc.scalar.activation(e[:, :, :], ps_s[:, :, :],
                                             AF.Exp, scale=SCALE)
                        state['e'] = e
                    def post(hp=hp, sub=sub, jsp=jsp, h=h):
                        ps_o = state[hp]
                        e = state['e']
                        if pv8:
                            nc.tensor.matmul(
                                ps_o[0:HD + 1, sub, :],
                                vext[:, jsp * 2:jsp * 2 + 2, h, :],
                                e[:, :, :], perf_mode=DR,
                                start=(jsp == 0),
                                stop=(jsp == n_js // 2 - 1))
                        else:
                            for j in range(2):
                                js = jsp * 2 + j
                                nc.tensor.matmul(ps_o[0:HD + 1, sub, :],
                                                 vext[:, js, h, :],
                                                 e[:, j, :],
                                                 start=(js == 0),
                                                 stop=(js == n_js - 1))
                        if sub == 1 and jsp == n_js // 2 - 1:
                            if first_group:
                                nc.vector.tensor_copy(
                                    a_acc[0:HD + 1, 2 * hp:2 * hp + 2, :],
                                    ps_o[0:HD + 1, :, :])
                            else:
                                nc.vector.tensor_add(
                                    a_acc[0:HD + 1, 2 * hp:2 * hp + 2, :],
                                    a_acc[0:HD + 1, 2 * hp:2 * hp + 2, :],
                                    ps_o[0:HD + 1, :, :])
                    chains.append((pre, post))
        return chains

    def run_interleaved(chains, fillers):
        """Emit chains, distributing filler closures between each chain's
        pre (scores+exp) and post (PV) to hide the exp latency."""
        frac_acc = 0.0
        per = len(fillers) / max(1, len(chains))
        fi = 0
        for pre, post in chains:
            pre()
            frac_acc += per
            while frac_acc >= 1.0 and fi < len(fillers):
                fillers[fi]()
                fi += 1
                frac_acc -= 1.0
            post()
        while fi < len(fillers):
            fillers[fi]()
            fi += 1

    # ================= P0 + self-attention =================
    x1T = persist.tile([128, WC, T], BF16, tag="x1T", name="x1T")
    x2T = persist.tile([128, WC, T], BF16, tag="x2T", name="x2T")
    acc = persist.tile([HD + 1, NH, NQ], F32, tag="acc", name="acc")
    acc2 = persist.tile([HD + 1, NH, NQ], F32, tag="acc2", name="acc2")

    with tc.tile_pool(name="early", bufs=1) as early, \
         tc.tile_pool(name="sa_w", bufs=2) as sa_w, \
         tc.tile_pool(name="sa", bufs=1) as sa:
        qT = early.tile([128, WC, T], BF16, tag="qT", name="qT")
        peT = early.tile([128, WC, T], BF16, tag="peT", name="peT")
        for b in range(B_loc):
            q_tm = scratch.tile([128, NQC, W], F32R, tag="tmland", bufs=2,
                                name="q_tm")
            nc.sync.dma_start(
                q_tm[:], q_flat[b * NQ:(b + 1) * NQ, :].rearrange(
                    "(c p) w -> p c w", p=128).bitcast(F32R))
            p_tm = scratch.tile([128, NQC, W], F32R, tag="tmland", bufs=2,
                                name="p_tm")
            nc.scalar.dma_start(
                p_tm[:], pe_flat[b * NQ:(b + 1) * NQ, :].rearrange(
                    "(c p) w -> p c w", p=128).bitcast(F32R))
            for fc in range(WC):
                t0 = b * NQ
                transpose_group(
                    qT[:, fc, t0:t0 + NQ],
                    [q_tm[:, tcx, fc * 128:(fc + 1) * 128]
                     for tcx in range(NQC)])
                transpose_group(
                    peT[:, fc, t0:t0 + NQ],
                    [p_tm[:, tcx, fc * 128:(fc + 1) * 128]
                     for tcx in range(NQC)])
        qkT = sa.tile([128, WC, T], BF16, tag="bigA", name="qkT")
        for fc in range(WC):
            nc.vector.tensor_add(qkT[:, fc, :], qT[:, fc, :], peT[:, fc, :])

        # -------- SA projections (batch-merged, N=T) --------
        wqt = load_wT(sa_w, "wt", "sa_wq_t", dram['sa_wq'], W, W, bufs=2)
        qsaT = sa.tile([128, WC, T], BF16, tag="qsaT", name="qsaT")
        for oc in range(WC):
            ps = mm_ps.tile([128, T], F32, tag="mm", name="ps_q")
            gemm(ps[:, 0:T], wqt, oc, lambda ic: qkT[:, ic, :], WC)
            nc.vector.tensor_copy(qsaT[:, oc, :], ps[:, 0:T])
        wkt = load_wT(sa_w, "wt", "sa_wk_t", dram['sa_wk'], W, W, bufs=2)
        ksaT = sa.tile([128, WC, T], BF16, tag="ksaT", name="ksaT")
        for oc in range(WC):
            ps = mm_ps.tile([128, T], F32, tag="mm", name="ps_k")
            gemm(ps[:, 0:T], wkt, oc, lambda ic: qkT[:, ic, :], WC)
            nc.vector.tensor_copy(ksaT[:, oc, :], ps[:, 0:T])
        wvt = load_wT(sa_w, "wt", "sa_wv_t", dram['sa_wv'], W, W, bufs=2)
        vext_sa = sa.tile([128, TC, NH, HD + 1], BF16, tag="vext",
                          name="vext_sa")
        nc.vector.memset(vext_sa[:, :, :, HD], 1.0)
        for tcx in range(TC):
            for oh in range(W // 512):
                ps = mm_ps.tile([128, 512], F32, tag="mm", name="ps_v")
                for ic in range(WC):
                    nc.tensor.matmul(
                        ps[:, 0:512],
                        qT[:, ic, tcx * 128:(tcx + 1) * 128],
                        wvt[:, ic, oh * 512:(oh + 1) * 512],
                        start=(ic == 0), stop=(ic == WC - 1))
                nh0 = oh * (512 // HD)
                nc.scalar.activation(
                    vext_sa[:, tcx, nh0:nh0 + 512 // HD, 0:HD],
                    ps[:, 0:512].rearrange("p (h d) -> p h d", d=HD),
                    AF.Copy)
        wot, wot_units = load_wT_units(sa_w, "wt", "sa_wo_t", dram['sa_wo'],
                                       W, W, bufs=2)
        wqt2, wq2_units = load_wT_units(sa_w, "wt", "ca_wq_t",
                                        dram['ca_wq'], W, W, bufs=2)

        # -------- SA attention (batches interleaved) + O proj + LN1 --------
        osaT = sa.tile([128, WC, T], BF16, tag="bigA", name="osaT")
        accs = [acc, acc2]
        blists = [attn_chains(b, qsaT, ksaT[:, :, b * NQ:(b + 1) * NQ],
                              vext_sa[:, b * NQC:(b + 1) * NQC, :, :],
                              accs[b], NQC, True, sa)
                  for b in range(B_loc)]
        chains = [c for pair in zip(*blists) for c in pair]
        run_interleaved(chains, wot_units + wq2_units)
        for b in range(B_loc):
            normalize_batch(b, accs[b], osaT)
        x1pre = sa.tile([128, WC, T], BF16, tag="bigB", name="x1pre")
        for oc in range(WC):
            ps = mm_ps.tile([128, T], F32, tag="mm", name="ps_o")
            gemm(ps[:, 0:T], wot, oc, lambda ic: osaT[:, ic, :], WC)
            nc.vector.tensor_add(x1pre[:, oc, :], ps[:, 0:T], qT[:, oc, :])
        layernorm(lambda ic: x1pre[:, ic, :], WC, T,
                  cols['ln1_g'], cols['ln1_b'],
                  lambda ic: x1T[:, ic, :])

        # -------- CA Q projection (needs peT before it dies) --------
        x1pT = sa.tile([128, WC, T], BF16, tag="bigB", name="x1pT")
        for fc in range(WC):
            nc.vector.tensor_add(x1pT[:, fc, :], x1T[:, fc, :],
                                 peT[:, fc, :])
        q2T = persist.tile([128, WC, T], BF16, tag="q2T", name="q2T")
        for oc in range(WC):
            ps = mm_ps.tile([128, T], F32, tag="mm", name="ps_q2")
            gemm(ps[:, 0:T], wqt2, oc, lambda ic: x1pT[:, ic, :], WC)
            nc.vector.tensor_copy(q2T[:, oc, :], ps[:, 0:T])

    # ================= cross-attention =================
    with tc.tile_pool(name="ca_w", bufs=1) as ca_w, \
         tc.tile_pool(name="ca", bufs=1) as ca, \
         tc.tile_pool(name="ca_g", bufs=2) as ca_g:
        wkt2 = load_wT(ca_w, "wtk2", "ca_wk_t", dram['ca_wk'], W, W,
                       dtype=F8, scale=16.0)
        wvt2 = load_wT(ca_w, "wtv2", "ca_wv_t", dram['ca_wv'], W, W,
                       dtype=F8, scale=16.0)

        ocaT = ca.tile([128, WC, T], BF16, tag="ocaT", name="ocaT")

        def build_proj(b, g):
            """Allocate group tiles and return (k2T, vext, units) where each
            unit emits one slice of the K/V projection pipeline."""
            k2T = ca_g.tile([128, WC, GK], BF16, tag="k2T", name="k2T")
            vext = ca_g.tile([128, GJS, NH, HD + 1], F8, tag="vext",
                             name="vext_ca")
            mT = ca.tile([128, WC, GK], F8, tag="mT8", bufs=2, name="mT")
            pst = {}
            units = [lambda: nc.vector.memset(vext[:, :, :, HD], 4.0)]
            for half in range(GJS // 2):
                def u_dma(half=half):
                    m_tm = scratch.tile([128, 2, W], F32R, tag="tmland",
                                        bufs=2, name="m_tm")
                    tok0 = b * S + g * GK + half * 256
                    nc.sync.dma_start(
                        m_tm[:], m_flat[tok0:tok0 + 256, :].rearrange(
                            "(c p) w -> p c w", p=128).bitcast(F32R))
                    pst[half] = m_tm
                units.append(u_dma)
                for fc in range(WC):
                    def u_tp(half=half, fc=fc):
                        transpose_group(
                            mT[:, fc, half * 256:(half + 1) * 256],
                            [pst[half][:, tcx, fc * 128:(fc + 1) * 128]
                             for tcx in range(2)])
                    units.append(u_tp)
            for oc in range(WC):
                def u_k(oc=oc):
                    ps = mm_ps.tile([128, GK], F32, tag="mm", name="ps_k2")
                    for icp in range(WC // 2):
                        nc.tensor.matmul(
                            ps[:, 0:GK],
                            wkt2[:, 2 * icp:2 * icp + 2,
                                 oc * 128:(oc + 1) * 128],
                            mT[:, 2 * icp:2 * icp + 2, :], perf_mode=DR,
                            start=(icp == 0), stop=(icp == WC // 2 - 1))
                    nc.vector.tensor_scalar_mul(k2T[:, oc, :], ps[:, 0:GK],
                                                1.0 / 16.0)
                units.append(u_k)
            for tch in range(GJS):
                for oh in range(W // 512):
                    def u_v(tch=tch, oh=oh):
                        ps = mm_ps.tile([128, 512], F32, tag="mm",
                                        name="ps_v2")
                        for icp in range(WC // 2):
                            nc.tensor.matmul(
                                ps[:, 0:512],
                                mT[:, 2 * icp:2 * icp + 2,
                                   tch * 128:(tch + 1) * 128],
                                wvt2[:, 2 * icp:2 * icp + 2,
                                     oh * 512:(oh + 1) * 512], perf_mode=DR,
                                start=(icp == 0), stop=(icp == WC // 2 - 1))
                        nh0 = oh * (512 // HD)
                        nc.vector.tensor_scalar_mul(
                            vext[:, tch, nh0:nh0 + 512 // HD, 0:HD],
                            ps[:, 0:512].rearrange("p (h d) -> p h d",
                                                   d=HD), 0.25)
                    units.append(u_v)
            return k2T, vext, units

        pending = None
        for b in range(B_loc):
            for g in range(NG):
                k2T, vext, units = build_proj(b, g)
                if pending is None:
                    for u in units:
                        u()
                else:
                    pb, pg, pk2T, pvext = pending
                    ch = attn_chains(pb, q2T, pk2T, pvext, acc, GJS,
                                     pg == 0, ca, pv8=True)
                    run_interleaved(ch, units)
                    if pg == NG - 1:
                        normalize_batch(pb, acc, ocaT)
                pending = (b, g, k2T, vext)
        pb, pg, pk2T, pvext = pending
        ch = attn_chains(pb, q2T, pk2T, pvext, acc, GJS, pg == 0, ca,
                         pv8=True)
        run_interleaved(ch, [])
        normalize_batch(pb, acc, ocaT)

        # -------- CA O proj + LN2 --------
        wot2 = load_wT(ca_w, "wtk2", "ca_wo_t", dram['ca_wo'], W, W)
        x2pre = ca.tile([128, WC, 512], BF16, tag="mT", bufs=2, name="x2pre")
        for oc in range(WC):
            ps = mm_ps.tile([128, T], F32, tag="mm", name="ps_o2")
            gemm(ps[:, 0:T], wot2, oc, lambda ic: ocaT[:, ic, :], WC)
            nc.vector.tensor_add(x2pre[:, oc, :], ps[:, 0:T], x1T[:, oc, :])
        layernorm(lambda ic: x2pre[:, ic, :], WC, T,
                  cols['ln2_g'], cols['ln2_b'],
                  lambda ic: x2T[:, ic, :])

    # ================= FFN =================
    with tc.tile_pool(name="ffn", bufs=1) as ffn:
        hT = ffn.tile([128, MC, T], BF16, tag="hT", name="hT")
        for oc in range(MC):
            w1t = ffn.tile([128, WC, 128], BF16, tag="w1t", bufs=3,
                           name="w1t")
            wr = scratch.tile([128, W], F32R, tag="wland", bufs=3, name="wr1")
            nc.sync.dma_start(
                wr[:], dram['ffn_w1'][oc * 128:(oc + 1) * 128, :]
                .bitcast(F32R))
            for half in range(W // 512):
                transpose_group(
                    w1t[:, half * 4:(half + 1) * 4, :],
                    [wr[:, half * 512 + k * 128:half * 512 + (k + 1) * 128]
                     for k in range(4)])
            ps = mm_ps.tile([128, T], F32, tag="mm", name="ps_h")
            for ic in range(WC):
                nc.tensor.matmul(ps[:, 0:T], w1t[:, ic, :], x2T[:, ic, :],
                                 start=(ic == 0), stop=(ic == WC - 1))
            nc.scalar.activation(hT[:, oc, :], ps[:, 0:T], AF.Relu,
                                 bias=b1_col[:, oc:oc + 1])
        x3pre = ffn.tile([128, WC, T], F32, tag="x3pre", name="x3pre")
        ps_s3 = pv_ps.tile([1, T], F32, tag="pv", name="ps_s3")
        ps_q3 = pv_ps.tile([1, T], F32, tag="pv", name="ps_q3")
        for oc in range(WC):
            w2t = ffn.tile([128, MC, 128], BF16, tag="w2t", bufs=2,
                           name="w2t")
            for piece in range(MLP // 1024):
                wr = scratch.tile([128, 1024], F32R, tag="wland", bufs=3,
                                  name="wr2")
                nc.sync.dma_start(
                    wr[:], dram['ffn_w2'][oc * 128:(oc + 1) * 128,
                                          piece * 1024:(piece + 1) * 1024]
                    .bitcast(F32R))
                for hh in range(2):
                    half = piece * 2 + hh
                    transpose_group(
                        w2t[:, half * 4:(half + 1) * 4, :],
                        [wr[:, hh * 512 + k * 128:hh * 512 + (k + 1) * 128]
                         for k in range(4)])
            ps = mm_ps.tile([128, T], F32, tag="mm", name="ps_f")
            for ic in range(MC):
                nc.tensor.matmul(ps[:, 0:T], w2t[:, ic, :], hT[:, ic, :],
                                 start=(ic == 0), stop=(ic == MC - 1))
            tmp = scratch.tile([128, T], F32, tag="ftmp", name="f_tmp")
            nc.scalar.activation(tmp[:, 0:T], ps[:, 0:T], AF.Identity,
                                 bias=cols['ffn_b2'][:, oc:oc + 1])
            nc.vector.tensor_add(x3pre[:, oc, :], tmp[:, 0:T], x2T[:, oc, :])
            # fold LN3 partition sums into this loop (bf16 shadow for matmul)
            x3b = scratch.tile([128, T], BF16, tag="sq3", name="x3b")
            nc.vector.tensor_copy(x3b[:, 0:T], x3pre[:, oc, :])
            nc.tensor.matmul(ps_s3[0:1, :], ones_bf[:, 0:1], x3b[:, 0:T],
                             start=(oc == 0), stop=(oc == WC - 1))
            sq = scratch.tile([128, T], BF16, tag="sq3", name="sq3")
            nc.vector.tensor_mul(sq[:, 0:T], x3b[:, 0:T], x3b[:, 0:T])
            nc.tensor.matmul(ps_q3[0:1, :], ones_bf[:, 0:1], sq[:, 0:T],
                             start=(oc == 0), stop=(oc == WC - 1))
        x3T = ffn.tile([128, WC, T], F32, tag="x3T", name="x3T")
        layernorm(lambda ic: x3pre[:, ic, :], WC, T,
                  cols['ln3_g'], cols['ln3_b'],
                  lambda ic: x3T[:, ic, :], pre_sums=(ps_s3, ps_q3),
                  f32_apply=True)
        for tcx in range(TC):
            o_tm = ffn.tile([128, W], F32, tag="o_tm", bufs=1, name="o_tm")
            for g in range(WC // 4):
                pt = tp_ps.tile([128, 512], F32, tag="tp", name="pt_out")
                for k in range(4):
                    nc.tensor.transpose(
                        pt[:, k * 128:(k + 1) * 128],
                        x3T[:, g * 4 + k, tcx * 128:(tcx + 1) * 128],
                        ident[:])
                nc.vector.tensor_copy(o_tm[:, g * 512:(g + 1) * 512],
                                      pt[:, 0:512])
            nc.sync.dma_start(out_flat[tcx * 128:(tcx + 1) * 128, :], o_tm[:])

    return out_d


_PROGRAM_CACHE = {}


def _get_program(B_loc, NQ, S, W, NH, MLP, JC=256, repeat=1):
    key = (B_loc, NQ, S, W, NH, MLP, JC, repeat)
    if key not in _PROGRAM_CACHE:
        nc = bacc.Bacc("TRN2", target_bir_lowering=False, debug=False)
        with tile.TileContext(nc) as tc, \
             nc.allow_low_precision(reason="bf16 matmul pipeline"):
            for r in range(repeat):
                with ExitStack() as ctx:
                    build_decoder(nc, tc, ctx, B_loc, NQ, S, W, NH, MLP, JC,
                                  suffix=("" if r == 0 else f"_r{r}"))
        nc.compile()
        _PROGRAM_CACHE[key] = nc
    return _PROGRAM_CACHE[key]


def kernel(**inputs):
    B, NQ, W = inputs['query'].shape
    S = inputs['enc_mem'].shape[1]
    MLP = inputs['ffn_w1'].shape[0]
    NH = 16
    assert B % N_CORES == 0
    B_loc = B // N_CORES

    nc = _get_program(B_loc, NQ, S, W, NH, MLP)

    shard_names = {'query', 'enc_mem', 'out_pos_enc'}
    in_maps = []
    for c in range(N_CORES):
        m = {}
        for k, v in inputs.items():
            v = np.ascontiguousarray(np.asarray(v, dtype=np.float32))
            if k in shard_names:
                m[k] = np.ascontiguousarray(v[c * B_loc:(c + 1) * B_loc])
            else:
                m[k] = v
        in_maps.append(m)

    res = run_bass_kernel_spmd(nc, in_maps, list(range(N_CORES)))
    return np.concatenate([res.results[c]["out"] for c in range(N_CORES)],
                          axis=0)
